# revision 10
# baseline (speedup 1.0000x reference)
"""Causal single-head attention on 8 trn2 NeuronCores, data-parallel over batch.

Reference computation (per batch element b):
  Q = x_b @ Wq.T + bq ; K = x_b @ Wk.T + bk ; V = x_b @ Wv.T + bv    (S=2048, D=A=1024)
  out_b = softmax(causal(Q K^T / 32)) V

Sharding: x is (S, B=8, D); core c handles batch element c. No collectives.

Per-core kernel design (v3, fp8 DoubleRow datapath + bf16 island):
  - the whole datapath (Q/K/V projections, score matmuls, PV matmuls) runs in
    fp8(e4m3) with perf_mode=DoubleRow, which packs two 128-row contraction
    chunks per PE pass (~1.7x bf16 matmul throughput). All operands are stored
    as [128, 2, free] pair-tiles so both matmul operands present the 3D
    [Ki, 2, free] access pattern DoubleRow requires.
  - fp8 quantization noise in Q/K/V enters the output through the softmax
    average; it decays like 1/sqrt(attention window). Measured profile: rel
    err 5e-2 at q=0 falling below 7e-3 for q>=128. So query rows 0-127 (which
    attend to keys 0-127 only) are recomputed exactly in bf16 (the "island"):
    bf16 copies of W and xT[:, :128] produce exact K0/Q0/V0, a 128x128
    masked softmax and a tiny PV give rows 0-127. Cost ~12us of the ~175us
    PE budget; keeps max-norm rel err ~7e-3 (tolerance 2e-2).
  - attention runs in scores-transposed orientation ST[k, q] (stationary
    KT a-pair chunks, moving QT) so exp(ST) chunks are directly the
    stationary operand for PV and no transposes exist anywhere. The causal
    triangle is walked in 4 query-windows of 512; within a window each key
    chunk emits only its valid query columns (exact 53% triangle, no waste).
    The diagonal 128x128 sub-block gets a triangular -1e30 mask add before
    exp; exp(ST) chunks land in fp8 P pair-tiles.
  - PV contracts key pairs with DoubleRow; odd pair tails are handled by
    zero-filling the (never-written) dead 128x128 P region so every chain is
    pure DoubleRow.
  - softmax denominator: DVE accumulates the same fp8 P values into
    den[128, 512] (consistency: out = sum(P~ V)/sum(P~) cancels common-mode
    quantization), a tiny ones-matmul reduces the partition dim, and the
    reciprocal is folded into the PSUM->SBUF output evacuation scale.
  - exp has no row-max subtraction: scores are N(0, ~0.33), |s| < ~2.
  - software pipeline with a 1-window skew: ST(w+1) is emitted before PV(w)
    so the PE has independent work over the exp-latency seam.
"""
import numpy as np

S = 2048
D = 1024
A = 1024
B = 8
QW = 512            # attention query-window
NW = S // QW        # 4
KC = 128            # key-chunk
SCALE = 1.0 / 32.0  # 1/sqrt(A)
NEG = -1e30

_cache = {}


def _emit_body(nc, tc):
    import concourse.bass as bass
    import concourse.mybir as mybir

    f32 = mybir.dt.float32
    bf16 = mybir.dt.bfloat16
    fp8 = mybir.dt.float8e4
    AF = mybir.ActivationFunctionType
    DR = mybir.MatmulPerfMode.DoubleRow

    xT8, x0T = nc.tensors["xT8"], nc.tensors["x0T"]
    wq8, wk8, wv8 = nc.tensors["wq8"], nc.tensors["wk8"], nc.tensors["wv8"]
    wq16, wk16, wv16 = nc.tensors["wq16"], nc.tensors["wk16"], nc.tensors["wv16"]
    bq, bk, bv = nc.tensors["bq"], nc.tensors["bk"], nc.tensors["bv"]
    mask, ones2, out = nc.tensors["mask"], nc.tensors["ones2"], nc.tensors["out"]

    def bcast_ap(handle, n_part, n_free):
        ap = handle[:]
        return bass.AP(tensor=ap.tensor, offset=ap.offset, ap=[[0, n_part], [1, n_free]])

    with (
        tc.tile_pool(name="const", bufs=1) as cp,
        tc.tile_pool(name="kt", bufs=4) as ktp,
        tc.tile_pool(name="qt", bufs=4) as qtp,
        tc.tile_pool(name="v", bufs=8) as vp,
        tc.tile_pool(name="xt", bufs=4) as xtp,
        tc.tile_pool(name="w8", bufs=2) as wp,
        tc.tile_pool(name="w16", bufs=3) as wp16,
        tc.tile_pool(name="isl", bufs=1) as ip,
    ):
        kt = [ktp.tile([128, 2, S], fp8, tag="kt", name=f"kt{j}") for j in range(4)]
        qt = [qtp.tile([128, 2, S], fp8, tag="qt", name=f"qt{j}") for j in range(4)]
        v8 = [vp.tile([128, 2, A], fp8, tag="v", name=f"v{p}") for p in range(8)]
        xt = [xtp.tile([128, 2, S], fp8, tag="xt", name=f"xt{p}") for p in range(4)]

        # startup: narrow first wave (just the a=0 weight columns + first x
        # slice) so the first accumulation chain starts ASAP; K-weights on the
        # sync queue, x on the scalar queue (separate HWDGE pipelines)
        # tiny constants first (biases feed the first evacuations)
        bq_t = cp.tile([128, 8], f32, tag="bq")
        bk_t = cp.tile([128, 8], f32, tag="bk")
        for a in range(8):
            nc.gpsimd.dma_start(
                out=bk_t[:, a : a + 1],
                in_=bk[a * 128 : (a + 1) * 128].rearrange("(p one) -> p one", one=1),
            )
        wk8t = wp.tile([128, 8, A], fp8, tag="w", name="wk8t")
        for d in range(8):
            nc.sync.dma_start(
                out=wk8t[:, d, 0:128], in_=wk8[d * 128 : (d + 1) * 128, 0:128]
            )
            # x waves split across two queues to halve head latency
            eng = nc.scalar if d % 2 == 0 else nc.gpsimd
            eng.dma_start(
                out=xt[d // 2][:, d % 2, 0:512],
                in_=xT8[d * 128 : (d + 1) * 128, 0:512],
            )
        for d in range(8):
            nc.sync.dma_start(
                out=wk8t[:, d, 128:1024], in_=wk8[d * 128 : (d + 1) * 128, 128:1024]
            )
        for d in range(8):
            eng = nc.scalar if d % 2 == 0 else nc.gpsimd
            eng.dma_start(
                out=xt[d // 2][:, d % 2, 512:2048],
                in_=xT8[d * 128 : (d + 1) * 128, 512:2048],
            )

        for a in range(8):
            nc.gpsimd.dma_start(
                out=bq_t[:, a : a + 1],
                in_=bq[a * 128 : (a + 1) * 128].rearrange("(p one) -> p one", one=1),
            )
        bv_t = cp.tile([128, A], f32, tag="bv")
        nc.gpsimd.dma_start(out=bv_t, in_=bcast_ap(bv, 128, A))
        ones_t = cp.tile([128, 2], f32, tag="ones")
        nc.gpsimd.dma_start(out=ones_t, in_=bcast_ap(ones2, 128, 2))
        mask_t = cp.tile([128, 128], f32, tag="mask")
        nc.gpsimd.dma_start(out=mask_t, in_=mask[:, :])
        ones_b = cp.tile([128, 2], bf16, tag="onesb")
        nc.vector.tensor_copy(ones_b, ones_t)

        # island inputs (needed ~90us in; gpsimd + scalar queue tails)
        x0t = [ip.tile([128, 128], bf16, tag=f"x0_{d}", name=f"x0_{d}") for d in range(8)]
        for d in range(8):
            nc.gpsimd.dma_start(out=x0t[d], in_=x0T[d * 128 : (d + 1) * 128, :])
        wk16t = wp16.tile([128, 8, A], bf16, tag="w16", name="wk16t")
        wq16t = wp16.tile([128, 8, A], bf16, tag="w16", name="wq16t")
        wv16t = wp16.tile([128, 8, A], bf16, tag="w16", name="wv16t")
        for d in range(8):
            nc.gpsimd.dma_start(out=wk16t[:, d, :], in_=wk16[d * 128 : (d + 1) * 128, :])
        for d in range(8):
            nc.gpsimd.dma_start(out=wq16t[:, d, :], in_=wq16[d * 128 : (d + 1) * 128, :])
        for d in range(8):
            nc.scalar.dma_start(out=wv16t[:, d, :], in_=wv16[d * 128 : (d + 1) * 128, :])

        # ---- fp8 projections: 4 DoubleRow d-pair passes per PSUM chain ----
        proj_psp = tc.tile_pool(name="ps", bufs=4, space="PSUM")
        psp = proj_psp.__enter__()

        def proj_qk(w8t, bias_t, dest, skip0=0):
            # skip0: omit the first `skip0` s-columns (qt cols 0:128 are only
            # ever consumed by the bf16 island, never by the fp8 path)
            for s4 in range(4):
                slo = s4 * 512 + (skip0 if s4 == 0 else 0)
                shi = (s4 + 1) * 512
                for a in range(8):
                    ps = psp.tile([128, shi - slo], f32, tag="ps")
                    for p in range(4):
                        nc.tensor.matmul(
                            ps,
                            w8t[:, 2 * p : 2 * p + 2, a * 128 : (a + 1) * 128],
                            xt[p][:, :, slo:shi],
                            start=(p == 0),
                            stop=(p == 3),
                            perf_mode=DR,
                        )
                    nc.scalar.activation(
                        dest[a // 2][:, a % 2, slo:shi],
                        ps,
                        AF.Identity,
                        bias=bias_t[:, a : a + 1],
                    )

        proj_qk(wk8t, bk_t, kt)

        wq8t = wp.tile([128, 8, A], fp8, tag="w", name="wq8t")
        for d in range(8):
            nc.sync.dma_start(out=wq8t[:, d, :], in_=wq8[d * 128 : (d + 1) * 128, :])
        proj_qk(wq8t, bq_t, qt, skip0=128)

        wv8t = wp.tile([128, 8, A], fp8, tag="w", name="wv8t")
        for d in range(8):
            nc.sync.dma_start(out=wv8t[:, d, :], in_=wv8[d * 128 : (d + 1) * 128, :])
        for sc in range(16):
            for ah in range(2):
                ps = psp.tile([128, 512], f32, tag="ps")
                for p in range(4):
                    nc.tensor.matmul(
                        ps,
                        xt[p][:, :, sc * 128 : (sc + 1) * 128],
                        wv8t[:, 2 * p : 2 * p + 2, ah * 512 : (ah + 1) * 512],
                        start=(p == 0),
                        stop=(p == 3),
                        perf_mode=DR,
                    )
                nc.vector.tensor_add(
                    v8[sc // 2][:, sc % 2, ah * 512 : (ah + 1) * 512],
                    ps,
                    bv_t[:, ah * 512 : (ah + 1) * 512],
                )

        # ---- island projections (bf16, exact K0/Q0/V0 for rows 0-127) ----
        kt0 = [ip.tile([128, 128], bf16, tag=f"kt0_{a}", name=f"kt0_{a}") for a in range(8)]
        qt0 = [ip.tile([128, 128], bf16, tag=f"qt0_{a}", name=f"qt0_{a}") for a in range(8)]
        v0 = ip.tile([128, A], bf16, tag="v0")
        for a in range(8):
            ps = psp.tile([128, 128], f32, tag="psi", name=f"psk0_{a}")
            for d in range(8):
                nc.tensor.matmul(
                    ps, wk16t[:, d, a * 128 : (a + 1) * 128], x0t[d],
                    start=(d == 0), stop=(d == 7),
                )
            nc.scalar.activation(kt0[a], ps, AF.Identity, bias=bk_t[:, a : a + 1])
        for a in range(8):
            ps = psp.tile([128, 128], f32, tag="psi", name=f"psq0_{a}")
            for d in range(8):
                nc.tensor.matmul(
                    ps, wq16t[:, d, a * 128 : (a + 1) * 128], x0t[d],
                    start=(d == 0), stop=(d == 7),
                )
            nc.scalar.activation(qt0[a], ps, AF.Identity, bias=bq_t[:, a : a + 1])
        for ah in range(2):
            ps = psp.tile([128, 512], f32, tag="ps", name=f"psv0_{ah}")
            for d in range(8):
                nc.tensor.matmul(
                    ps, x0t[d], wv16t[:, d, ah * 512 : (ah + 1) * 512],
                    start=(d == 0), stop=(d == 7),
                )
            nc.vector.tensor_add(
                v0[:, ah * 512 : (ah + 1) * 512], ps, bv_t[:, ah * 512 : (ah + 1) * 512]
            )

        proj_psp.__exit__(None, None, None)

        # ---- attention ----
        with (
            tc.tile_pool(name="p8", bufs=16) as pp,
            tc.tile_pool(name="stt", bufs=3) as sttp,
            tc.tile_pool(name="den", bufs=2) as dnp,
            tc.tile_pool(name="ob", bufs=3) as obp,
            tc.tile_pool(name="rin", bufs=6) as rp,
            tc.tile_pool(name="pst", bufs=4, space="PSUM") as pstp,
            tc.tile_pool(name="po", bufs=3, space="PSUM") as pop,
            tc.tile_pool(name="pd", bufs=1, space="PSUM") as pdp,
        ):
            def island_scores():
                ps = pstp.tile([128, 128], f32, tag="st", name="ps0")
                for a in range(8):
                    nc.tensor.matmul(ps, kt0[a], qt0[a], start=(a == 0), stop=(a == 7))
                stt = sttp.tile([128, 128], f32, tag="stt", name="stt0")
                nc.vector.tensor_add(stt, ps, mask_t)
                p0 = ip.tile([128, 128], bf16, tag="p0")
                nc.scalar.activation(p0, stt, AF.Exp, scale=SCALE)
                return p0

            def island_pv(p0):
                pd = pdp.tile([128, 2], f32, tag="pd", name="pd0")
                nc.tensor.matmul(pd, p0, ones_b, start=True, stop=True)
                rinv = rp.tile([128, 1], f32, tag="rinv", name="rinv0")
                nc.vector.reciprocal(rinv, pd[:, 0:1])
                osb = obp.tile([128, A], f32, tag="ob", name="osb0")
                for ah in range(2):
                    po = pop.tile([128, 512], f32, tag="po", name=f"po0_{ah}")
                    nc.tensor.matmul(
                        po, p0, v0[:, ah * 512 : (ah + 1) * 512], start=True, stop=True
                    )
                    nc.scalar.activation(
                        osb[:, ah * 512 : (ah + 1) * 512], po, AF.Copy, scale=rinv
                    )
                nc.sync.dma_start(out=out[0:128, :], in_=osb)

            def emit_st(w):
                """Score chunks + exp + denominator partials for one q-window."""
                kcmax = 4 * (w + 1)
                den = dnp.tile([128, QW], f32, tag="den", name=f"den{w}")
                nc.gpsimd.memset(den, 0.0)
                pw = [
                    pp.tile([128, 2, QW], fp8, tag="p8", name=f"p{w}_{j}")
                    for j in range(kcmax // 2)
                ]
                for kc in range(kcmax):
                    qlo_g = max(w * QW, kc * KC)
                    if w == 0 and kc == 0:
                        qlo_g = 128  # rows 0-127 belong to the bf16 island
                    ql = qlo_g - w * QW
                    nq = QW - ql
                    ps = pstp.tile([128, nq], f32, tag="st", name=f"st{w}_{kc}")
                    if nq <= 256:
                        # narrow block: DoubleRow is LDWEIGHTS-bound here, plain
                        # fp8 128-contraction MMs with FWL weight loads win
                        for j in range(4):
                            for i in range(2):
                                nc.tensor.matmul(
                                    ps,
                                    kt[j][:, i, kc * KC : (kc + 1) * KC],
                                    qt[j][:, i, qlo_g : (w + 1) * QW],
                                    start=(j == 0 and i == 0),
                                    stop=(j == 3 and i == 1),
                                )
                    else:
                        for j in range(4):
                            nc.tensor.matmul(
                                ps,
                                kt[j][:, :, kc * KC : (kc + 1) * KC],
                                qt[j][:, :, qlo_g : (w + 1) * QW],
                                start=(j == 0),
                                stop=(j == 3),
                                perf_mode=DR,
                            )
                    sub = kc % 2
                    pwt = pw[kc // 2]
                    if kc // 4 == w and kc != 0:
                        # diagonal 128x128 sub-block: triangular causal mask
                        stt = sttp.tile([128, 128], f32, tag="stt", name=f"stt{w}_{kc}")
                        nc.vector.tensor_add(stt, ps[:, 0:128], mask_t)
                        nc.scalar.activation(
                            pwt[:, sub, ql : ql + 128], stt, AF.Exp, scale=SCALE
                        )
                        if nq > 128:
                            nc.scalar.activation(
                                pwt[:, sub, ql + 128 : QW], ps[:, 128:nq],
                                AF.Exp, scale=SCALE,
                            )
                    else:
                        nc.scalar.activation(pwt[:, sub, ql:QW], ps, AF.Exp, scale=SCALE)
                    nc.vector.tensor_add(
                        den[:, ql:QW], den[:, ql:QW], pwt[:, sub, ql:QW]
                    )
                return pw, den

            def emit_pv(w, pw, den):
                """Denominator reduction + PV chains + evacuation for one q-window."""
                for qc in range(4 * w, 4 * w + 4):
                    if qc == 0:
                        continue
                    ql = (qc % 4) * 128
                    nfull = (qc + 1) // 2  # full k-pairs
                    odd = (qc + 1) % 2     # trailing single k-chunk
                    pd = pdp.tile([128, 2], f32, tag="pd", name=f"pd{qc}")
                    nc.tensor.matmul(
                        pd, den[:, ql : ql + 128], ones_t, start=True, stop=True
                    )
                    rinv = rp.tile([128, 1], f32, tag="rinv", name=f"rinv{qc}")
                    nc.vector.reciprocal(rinv, pd[:, 0:1])
                    osb = obp.tile([128, A], f32, tag="ob", name=f"ob{qc}")
                    for ah in range(2):
                        po = pop.tile([128, 512], f32, tag="po", name=f"po{qc}_{ah}")
                        for p in range(nfull):
                            nc.tensor.matmul(
                                po,
                                pw[p][:, :, ql : ql + 128],
                                v8[p][:, :, ah * 512 : (ah + 1) * 512],
                                start=(p == 0),
                                stop=(p == nfull - 1 and not odd),
                                perf_mode=DR,
                            )
                        if odd:
                            nc.tensor.matmul(
                                po,
                                pw[nfull][:, 0, ql : ql + 128],
                                v8[nfull][:, 0, ah * 512 : (ah + 1) * 512],
                                start=(nfull == 0),
                                stop=True,
                            )
                        nc.scalar.activation(
                            osb[:, ah * 512 : (ah + 1) * 512], po, AF.Copy, scale=rinv
                        )
                        oeng = nc.sync if ah == 0 else nc.gpsimd
                        oeng.dma_start(
                            out=out[qc * 128 : (qc + 1) * 128, ah * 512 : (ah + 1) * 512],
                            in_=osb[:, ah * 512 : (ah + 1) * 512],
                        )

            # software pipeline with a 1-window skew: ST(w+1) is emitted before
            # PV(w) so the PE has independent work over the exp-latency seam
            p0 = island_scores()
            prev = emit_st(0)
            island_pv(p0)
            for w in range(1, NW):
                st_state = emit_st(w)
                emit_pv(w - 1, *prev)
                prev = st_state
            emit_pv(NW - 1, *prev)


def _build(repeat=1):
    from concourse import bacc
    import concourse.mybir as mybir
    import concourse.tile as tile

    f32 = mybir.dt.float32
    bf16 = mybir.dt.bfloat16
    fp8 = mybir.dt.float8e4

    nc = bacc.Bacc("TRN2", target_bir_lowering=False)
    nc.tensors = {}
    nc.tensors["xT8"] = nc.dram_tensor("xT8", [D, S], fp8, kind="ExternalInput")
    nc.tensors["x0T"] = nc.dram_tensor("x0T", [D, 128], bf16, kind="ExternalInput")
    nc.tensors["wq8"] = nc.dram_tensor("wq8", [D, A], fp8, kind="ExternalInput")
    nc.tensors["wk8"] = nc.dram_tensor("wk8", [D, A], fp8, kind="ExternalInput")
    nc.tensors["wv8"] = nc.dram_tensor("wv8", [D, A], fp8, kind="ExternalInput")
    nc.tensors["wq16"] = nc.dram_tensor("wq16", [D, A], bf16, kind="ExternalInput")
    nc.tensors["wk16"] = nc.dram_tensor("wk16", [D, A], bf16, kind="ExternalInput")
    nc.tensors["wv16"] = nc.dram_tensor("wv16", [D, A], bf16, kind="ExternalInput")
    nc.tensors["bq"] = nc.dram_tensor("bq", [A], f32, kind="ExternalInput")
    nc.tensors["bk"] = nc.dram_tensor("bk", [A], f32, kind="ExternalInput")
    nc.tensors["bv"] = nc.dram_tensor("bv", [A], f32, kind="ExternalInput")
    nc.tensors["mask"] = nc.dram_tensor("mask", [128, 128], f32, kind="ExternalInput")
    nc.tensors["ones2"] = nc.dram_tensor("ones2", [2], f32, kind="ExternalInput")
    nc.tensors["out"] = nc.dram_tensor("out", [S, A], f32, kind="ExternalOutput")

    with tile.TileContext(nc) as tc:
        if repeat > 1:
            with tc.For_i(0, repeat, 1):
                _emit_body(nc, tc)
        else:
            _emit_body(nc, tc)

    nc.finalize()
    return nc


def _prep_in_maps(x, Wq, bq, Wk, bk, Wv, bv):
    """Build per-core input maps (host-side shard + layout/dtype transforms)."""
    import ml_dtypes

    bf = ml_dtypes.bfloat16
    f8 = ml_dtypes.float8_e4m3  # TRN FP8_EXP4 (max +-240, has inf) == this dtype
    x = np.asarray(x, dtype=np.float32)
    wqT = np.ascontiguousarray(np.asarray(Wq, dtype=np.float32).T)
    wkT = np.ascontiguousarray(np.asarray(Wk, dtype=np.float32).T)
    wvT = np.ascontiguousarray(np.asarray(Wv, dtype=np.float32).T)
    bq = np.asarray(bq, dtype=np.float32)
    bk = np.asarray(bk, dtype=np.float32)
    bv = np.asarray(bv, dtype=np.float32)
    kq = np.arange(128)
    mask = np.where(kq[:, None] <= kq[None, :], 0.0, NEG).astype(np.float32)
    ones2 = np.ones(2, dtype=np.float32)
    shared = {
        "wq8": wqT.astype(f8), "wk8": wkT.astype(f8), "wv8": wvT.astype(f8),
        "wq16": wqT.astype(bf), "wk16": wkT.astype(bf), "wv16": wvT.astype(bf),
        "bq": bq, "bk": bk, "bv": bv, "mask": mask, "ones2": ones2,
    }
    in_maps = []
    for c in range(B):
        xTc = np.ascontiguousarray(x[:, c, :].T)  # (D, S) f32
        in_maps.append(
            {"xT8": xTc.astype(f8), "x0T": xTc[:, 0:128].astype(bf), **shared}
        )
    return in_maps


def get_nc(repeat=1):
    key = ("nc", repeat)
    if key not in _cache:
        _cache[key] = _build(repeat)
    return _cache[key]


def kernel(x, Wq, bq, Wk, bk, Wv, bv):
    from concourse.bass_utils import run_bass_kernel_spmd

    nc = get_nc()
    in_maps = _prep_in_maps(x, Wq, bq, Wk, bk, Wv, bv)
    res = run_bass_kernel_spmd(nc, in_maps, core_ids=list(range(B)))
    outs = np.stack([res.results[c]["out"] for c in range(B)], axis=0)  # (B, S, A)
    return np.ascontiguousarray(outs.transpose(1, 0, 2))  # (S, B, A)


# revision 11
# speedup vs baseline: 1.0101x; 1.0101x over previous
"""Causal single-head attention on 8 trn2 NeuronCores, data-parallel over batch.

Reference computation (per batch element b):
  Q = x_b @ Wq.T + bq ; K = x_b @ Wk.T + bk ; V = x_b @ Wv.T + bv    (S=2048, D=A=1024)
  out_b = softmax(causal(Q K^T / 32)) V

Sharding: x is (S, B=8, D); core c handles batch element c. No collectives.

Per-core kernel design (v3, fp8 DoubleRow datapath + bf16 island):
  - the whole datapath (Q/K/V projections, score matmuls, PV matmuls) runs in
    fp8(e4m3) with perf_mode=DoubleRow, which packs two 128-row contraction
    chunks per PE pass (~1.7x bf16 matmul throughput). All operands are stored
    as [128, 2, free] pair-tiles so both matmul operands present the 3D
    [Ki, 2, free] access pattern DoubleRow requires.
  - fp8 quantization noise in Q/K/V enters the output through the softmax
    average; it decays like 1/sqrt(attention window). Measured profile: rel
    err 5e-2 at q=0 falling below 7e-3 for q>=128. So query rows 0-127 (which
    attend to keys 0-127 only) are recomputed exactly in bf16 (the "island"):
    bf16 copies of W and xT[:, :128] produce exact K0/Q0/V0, a 128x128
    masked softmax and a tiny PV give rows 0-127. Cost ~12us of the ~175us
    PE budget; keeps max-norm rel err ~7e-3 (tolerance 2e-2).
  - attention runs in scores-transposed orientation ST[k, q] (stationary
    KT a-pair chunks, moving QT) so exp(ST) chunks are directly the
    stationary operand for PV and no transposes exist anywhere. The causal
    triangle is walked in 4 query-windows of 512; within a window each key
    chunk emits only its valid query columns (exact 53% triangle, no waste).
    The diagonal 128x128 sub-block gets a triangular -1e30 mask add before
    exp; exp(ST) chunks land in fp8 P pair-tiles.
  - PV contracts key pairs with DoubleRow; odd pair tails are handled by
    zero-filling the (never-written) dead 128x128 P region so every chain is
    pure DoubleRow.
  - softmax denominator: DVE accumulates the same fp8 P values into
    den[128, 512] (consistency: out = sum(P~ V)/sum(P~) cancels common-mode
    quantization), a tiny ones-matmul reduces the partition dim, and the
    reciprocal is folded into the PSUM->SBUF output evacuation scale.
  - exp has no row-max subtraction: scores are N(0, ~0.33), |s| < ~2.
  - software pipeline with a 1-window skew: ST(w+1) is emitted before PV(w)
    so the PE has independent work over the exp-latency seam.
"""
import numpy as np

S = 2048
D = 1024
A = 1024
B = 8
QW = 512            # attention query-window
NW = S // QW        # 4
KC = 128            # key-chunk
SCALE = 1.0 / 32.0  # 1/sqrt(A)
NEG = -1e30

_cache = {}


def _emit_body(nc, tc):
    import concourse.bass as bass
    import concourse.mybir as mybir

    f32 = mybir.dt.float32
    bf16 = mybir.dt.bfloat16
    fp8 = mybir.dt.float8e4
    AF = mybir.ActivationFunctionType
    DR = mybir.MatmulPerfMode.DoubleRow

    xT8, x0T = nc.tensors["xT8"], nc.tensors["x0T"]
    wq8, wk8, wv8 = nc.tensors["wq8"], nc.tensors["wk8"], nc.tensors["wv8"]
    wq16, wk16, wv16 = nc.tensors["wq16"], nc.tensors["wk16"], nc.tensors["wv16"]
    bq, bk, bv = nc.tensors["bq"], nc.tensors["bk"], nc.tensors["bv"]
    mask, ones2, out = nc.tensors["mask"], nc.tensors["ones2"], nc.tensors["out"]

    def bcast_ap(handle, n_part, n_free):
        ap = handle[:]
        return bass.AP(tensor=ap.tensor, offset=ap.offset, ap=[[0, n_part], [1, n_free]])

    with (
        tc.tile_pool(name="const", bufs=1) as cp,
        tc.tile_pool(name="kt", bufs=4) as ktp,
        tc.tile_pool(name="qt", bufs=4) as qtp,
        tc.tile_pool(name="v", bufs=8) as vp,
        tc.tile_pool(name="xt", bufs=4) as xtp,
        tc.tile_pool(name="w8", bufs=2) as wp,
        tc.tile_pool(name="w16", bufs=3) as wp16,
        tc.tile_pool(name="isl", bufs=1) as ip,
    ):
        kt = [ktp.tile([128, 2, S], fp8, tag="kt", name=f"kt{j}") for j in range(4)]
        qt = [qtp.tile([128, 2, S], fp8, tag="qt", name=f"qt{j}") for j in range(4)]
        v8 = [vp.tile([128, 2, A], fp8, tag="v", name=f"v{p}") for p in range(8)]
        xt = [xtp.tile([128, 2, S], fp8, tag="xt", name=f"xt{p}") for p in range(4)]

        # startup: narrow first wave (just the a=0 weight columns + first x
        # slice) so the first accumulation chain starts ASAP; K-weights on the
        # sync queue, x on the scalar queue (separate HWDGE pipelines)
        # tiny constants first (biases feed the first evacuations)
        bq_t = cp.tile([128, 8], f32, tag="bq")
        bk_t = cp.tile([128, 8], f32, tag="bk")
        for a in range(8):
            nc.gpsimd.dma_start(
                out=bk_t[:, a : a + 1],
                in_=bk[a * 128 : (a + 1) * 128].rearrange("(p one) -> p one", one=1),
            )
        wk8t = wp.tile([128, 8, A], fp8, tag="w", name="wk8t")
        for d in range(8):
            nc.sync.dma_start(
                out=wk8t[:, d, 0:128], in_=wk8[d * 128 : (d + 1) * 128, 0:128]
            )
            # x waves split across two queues to halve head latency
            eng = nc.scalar if d % 2 == 0 else nc.gpsimd
            eng.dma_start(
                out=xt[d // 2][:, d % 2, 0:512],
                in_=xT8[d * 128 : (d + 1) * 128, 0:512],
            )
        for d in range(8):
            nc.sync.dma_start(
                out=wk8t[:, d, 128:1024], in_=wk8[d * 128 : (d + 1) * 128, 128:1024]
            )
        for d in range(8):
            eng = nc.scalar if d % 2 == 0 else nc.gpsimd
            eng.dma_start(
                out=xt[d // 2][:, d % 2, 512:2048],
                in_=xT8[d * 128 : (d + 1) * 128, 512:2048],
            )

        for a in range(8):
            nc.gpsimd.dma_start(
                out=bq_t[:, a : a + 1],
                in_=bq[a * 128 : (a + 1) * 128].rearrange("(p one) -> p one", one=1),
            )
        bv_t = cp.tile([128, A], f32, tag="bv")
        nc.gpsimd.dma_start(out=bv_t, in_=bcast_ap(bv, 128, A))
        ones_t = cp.tile([128, 2], f32, tag="ones")
        nc.gpsimd.dma_start(out=ones_t, in_=bcast_ap(ones2, 128, 2))
        mask_t = cp.tile([128, 128], f32, tag="mask")
        nc.gpsimd.dma_start(out=mask_t, in_=mask[:, :])
        ones_b = cp.tile([128, 2], bf16, tag="onesb")
        nc.vector.tensor_copy(ones_b, ones_t)

        # island inputs (needed ~90us in; gpsimd + scalar queue tails)
        x0t = [ip.tile([128, 128], bf16, tag=f"x0_{d}", name=f"x0_{d}") for d in range(8)]
        for d in range(8):
            nc.gpsimd.dma_start(out=x0t[d], in_=x0T[d * 128 : (d + 1) * 128, :])
        wk16t = wp16.tile([128, 8, A], bf16, tag="w16", name="wk16t")
        wq16t = wp16.tile([128, 8, A], bf16, tag="w16", name="wq16t")
        wv16t = wp16.tile([128, 8, A], bf16, tag="w16", name="wv16t")
        for d in range(8):
            nc.gpsimd.dma_start(out=wk16t[:, d, :], in_=wk16[d * 128 : (d + 1) * 128, :])
        for d in range(8):
            nc.gpsimd.dma_start(out=wq16t[:, d, :], in_=wq16[d * 128 : (d + 1) * 128, :])
        for d in range(8):
            nc.scalar.dma_start(out=wv16t[:, d, :], in_=wv16[d * 128 : (d + 1) * 128, :])

        # ---- fp8 projections: 4 DoubleRow d-pair passes per PSUM chain ----
        proj_psp = tc.tile_pool(name="ps", bufs=4, space="PSUM")
        psp = proj_psp.__enter__()

        def proj_qk(w8t, bias_t, dest, skip0=0):
            # skip0: omit the first `skip0` s-columns (qt cols 0:128 are only
            # ever consumed by the bf16 island, never by the fp8 path)
            for s4 in range(4):
                slo = s4 * 512 + (skip0 if s4 == 0 else 0)
                shi = (s4 + 1) * 512
                for a in range(8):
                    ps = psp.tile([128, shi - slo], f32, tag="ps")
                    for p in range(4):
                        nc.tensor.matmul(
                            ps,
                            w8t[:, 2 * p : 2 * p + 2, a * 128 : (a + 1) * 128],
                            xt[p][:, :, slo:shi],
                            start=(p == 0),
                            stop=(p == 3),
                            perf_mode=DR,
                        )
                    nc.scalar.activation(
                        dest[a // 2][:, a % 2, slo:shi],
                        ps,
                        AF.Identity,
                        bias=bias_t[:, a : a + 1],
                    )

        proj_qk(wk8t, bk_t, kt)

        wq8t = wp.tile([128, 8, A], fp8, tag="w", name="wq8t")
        for d in range(8):
            nc.sync.dma_start(out=wq8t[:, d, :], in_=wq8[d * 128 : (d + 1) * 128, :])
        proj_qk(wq8t, bq_t, qt)

        wv8t = wp.tile([128, 8, A], fp8, tag="w", name="wv8t")
        for d in range(8):
            nc.sync.dma_start(out=wv8t[:, d, :], in_=wv8[d * 128 : (d + 1) * 128, :])
        for sc in range(16):
            for ah in range(2):
                ps = psp.tile([128, 512], f32, tag="ps")
                for p in range(4):
                    nc.tensor.matmul(
                        ps,
                        xt[p][:, :, sc * 128 : (sc + 1) * 128],
                        wv8t[:, 2 * p : 2 * p + 2, ah * 512 : (ah + 1) * 512],
                        start=(p == 0),
                        stop=(p == 3),
                        perf_mode=DR,
                    )
                nc.vector.tensor_add(
                    v8[sc // 2][:, sc % 2, ah * 512 : (ah + 1) * 512],
                    ps,
                    bv_t[:, ah * 512 : (ah + 1) * 512],
                )

        # ---- island projections (bf16, exact K0/Q0/V0 for rows 0-127) ----
        kt0 = [ip.tile([128, 128], bf16, tag=f"kt0_{a}", name=f"kt0_{a}") for a in range(8)]
        qt0 = [ip.tile([128, 128], bf16, tag=f"qt0_{a}", name=f"qt0_{a}") for a in range(8)]
        v0 = ip.tile([128, A], bf16, tag="v0")
        for a in range(8):
            ps = psp.tile([128, 128], f32, tag="psi", name=f"psk0_{a}")
            for d in range(8):
                nc.tensor.matmul(
                    ps, wk16t[:, d, a * 128 : (a + 1) * 128], x0t[d],
                    start=(d == 0), stop=(d == 7),
                )
            nc.scalar.activation(kt0[a], ps, AF.Identity, bias=bk_t[:, a : a + 1])
        for a in range(8):
            ps = psp.tile([128, 128], f32, tag="psi", name=f"psq0_{a}")
            for d in range(8):
                nc.tensor.matmul(
                    ps, wq16t[:, d, a * 128 : (a + 1) * 128], x0t[d],
                    start=(d == 0), stop=(d == 7),
                )
            nc.scalar.activation(qt0[a], ps, AF.Identity, bias=bq_t[:, a : a + 1])
        for ah in range(2):
            ps = psp.tile([128, 512], f32, tag="ps", name=f"psv0_{ah}")
            for d in range(8):
                nc.tensor.matmul(
                    ps, x0t[d], wv16t[:, d, ah * 512 : (ah + 1) * 512],
                    start=(d == 0), stop=(d == 7),
                )
            nc.vector.tensor_add(
                v0[:, ah * 512 : (ah + 1) * 512], ps, bv_t[:, ah * 512 : (ah + 1) * 512]
            )

        proj_psp.__exit__(None, None, None)

        # ---- attention ----
        with (
            tc.tile_pool(name="p8", bufs=16) as pp,
            tc.tile_pool(name="stt", bufs=3) as sttp,
            tc.tile_pool(name="den", bufs=2) as dnp,
            tc.tile_pool(name="ob", bufs=3) as obp,
            tc.tile_pool(name="rin", bufs=6) as rp,
            tc.tile_pool(name="pst", bufs=4, space="PSUM") as pstp,
            tc.tile_pool(name="po", bufs=3, space="PSUM") as pop,
            tc.tile_pool(name="pd", bufs=1, space="PSUM") as pdp,
        ):
            def island_scores():
                ps = pstp.tile([128, 128], f32, tag="st", name="ps0")
                for a in range(8):
                    nc.tensor.matmul(ps, kt0[a], qt0[a], start=(a == 0), stop=(a == 7))
                stt = sttp.tile([128, 128], f32, tag="stt", name="stt0")
                nc.vector.tensor_add(stt, ps, mask_t)
                p0 = ip.tile([128, 128], bf16, tag="p0")
                nc.scalar.activation(p0, stt, AF.Exp, scale=SCALE)
                return p0

            def island_pv(p0):
                pd = pdp.tile([128, 2], f32, tag="pd", name="pd0")
                nc.tensor.matmul(pd, p0, ones_b, start=True, stop=True)
                rinv = rp.tile([128, 1], f32, tag="rinv", name="rinv0")
                nc.vector.reciprocal(rinv, pd[:, 0:1])
                osb = obp.tile([128, A], f32, tag="ob", name="osb0")
                for ah in range(2):
                    po = pop.tile([128, 512], f32, tag="po", name=f"po0_{ah}")
                    nc.tensor.matmul(
                        po, p0, v0[:, ah * 512 : (ah + 1) * 512], start=True, stop=True
                    )
                    nc.scalar.activation(
                        osb[:, ah * 512 : (ah + 1) * 512], po, AF.Copy, scale=rinv
                    )
                nc.sync.dma_start(out=out[0:128, :], in_=osb)

            def emit_st(w):
                """Score chunks + exp + denominator partials for one q-window."""
                kcmax = 4 * (w + 1)
                den = dnp.tile([128, QW], f32, tag="den", name=f"den{w}")
                nc.gpsimd.memset(den, 0.0)
                pw = [
                    pp.tile([128, 2, QW], fp8, tag="p8", name=f"p{w}_{j}")
                    for j in range(kcmax // 2)
                ]
                for kc in range(kcmax):
                    qlo_g = max(w * QW, kc * KC)
                    if w == 0 and kc == 0:
                        qlo_g = 128  # rows 0-127 belong to the bf16 island
                    ql = qlo_g - w * QW
                    nq = QW - ql
                    ps = pstp.tile([128, nq], f32, tag="st", name=f"st{w}_{kc}")
                    if nq <= 256:
                        # narrow block: DoubleRow is LDWEIGHTS-bound here, plain
                        # fp8 128-contraction MMs with FWL weight loads win
                        for j in range(4):
                            for i in range(2):
                                nc.tensor.matmul(
                                    ps,
                                    kt[j][:, i, kc * KC : (kc + 1) * KC],
                                    qt[j][:, i, qlo_g : (w + 1) * QW],
                                    start=(j == 0 and i == 0),
                                    stop=(j == 3 and i == 1),
                                )
                    else:
                        for j in range(4):
                            nc.tensor.matmul(
                                ps,
                                kt[j][:, :, kc * KC : (kc + 1) * KC],
                                qt[j][:, :, qlo_g : (w + 1) * QW],
                                start=(j == 0),
                                stop=(j == 3),
                                perf_mode=DR,
                            )
                    sub = kc % 2
                    pwt = pw[kc // 2]
                    if kc // 4 == w and kc != 0:
                        # diagonal 128x128 sub-block: triangular causal mask
                        stt = sttp.tile([128, 128], f32, tag="stt", name=f"stt{w}_{kc}")
                        nc.vector.tensor_add(stt, ps[:, 0:128], mask_t)
                        nc.scalar.activation(
                            pwt[:, sub, ql : ql + 128], stt, AF.Exp, scale=SCALE
                        )
                        if nq > 128:
                            nc.scalar.activation(
                                pwt[:, sub, ql + 128 : QW], ps[:, 128:nq],
                                AF.Exp, scale=SCALE,
                            )
                    else:
                        nc.scalar.activation(pwt[:, sub, ql:QW], ps, AF.Exp, scale=SCALE)
                    nc.vector.tensor_add(
                        den[:, ql:QW], den[:, ql:QW], pwt[:, sub, ql:QW]
                    )
                return pw, den

            def emit_pv(w, pw, den):
                """Denominator reduction + PV chains + evacuation for one q-window."""
                for qc in range(4 * w, 4 * w + 4):
                    if qc == 0:
                        continue
                    ql = (qc % 4) * 128
                    nfull = (qc + 1) // 2  # full k-pairs
                    odd = (qc + 1) % 2     # trailing single k-chunk
                    pd = pdp.tile([128, 2], f32, tag="pd", name=f"pd{qc}")
                    nc.tensor.matmul(
                        pd, den[:, ql : ql + 128], ones_t, start=True, stop=True
                    )
                    rinv = rp.tile([128, 1], f32, tag="rinv", name=f"rinv{qc}")
                    nc.vector.reciprocal(rinv, pd[:, 0:1])
                    osb = obp.tile([128, A], f32, tag="ob", name=f"ob{qc}")
                    for ah in range(2):
                        po = pop.tile([128, 512], f32, tag="po", name=f"po{qc}_{ah}")
                        for p in range(nfull):
                            nc.tensor.matmul(
                                po,
                                pw[p][:, :, ql : ql + 128],
                                v8[p][:, :, ah * 512 : (ah + 1) * 512],
                                start=(p == 0),
                                stop=(p == nfull - 1 and not odd),
                                perf_mode=DR,
                            )
                        if odd:
                            nc.tensor.matmul(
                                po,
                                pw[nfull][:, 0, ql : ql + 128],
                                v8[nfull][:, 0, ah * 512 : (ah + 1) * 512],
                                start=(nfull == 0),
                                stop=True,
                            )
                        nc.scalar.activation(
                            osb[:, ah * 512 : (ah + 1) * 512], po, AF.Copy, scale=rinv
                        )
                        nc.sync.dma_start(
                            out=out[qc * 128 : (qc + 1) * 128, ah * 512 : (ah + 1) * 512],
                            in_=osb[:, ah * 512 : (ah + 1) * 512],
                        )

            # software pipeline with a 1-window skew: ST(w+1) is emitted before
            # PV(w) so the PE has independent work over the exp-latency seam
            p0 = island_scores()
            prev = emit_st(0)
            island_pv(p0)
            for w in range(1, NW):
                st_state = emit_st(w)
                emit_pv(w - 1, *prev)
                prev = st_state
            emit_pv(NW - 1, *prev)


def _build(repeat=1):
    from concourse import bacc
    import concourse.mybir as mybir
    import concourse.tile as tile

    f32 = mybir.dt.float32
    bf16 = mybir.dt.bfloat16
    fp8 = mybir.dt.float8e4

    nc = bacc.Bacc("TRN2", target_bir_lowering=False)
    nc.tensors = {}
    nc.tensors["xT8"] = nc.dram_tensor("xT8", [D, S], fp8, kind="ExternalInput")
    nc.tensors["x0T"] = nc.dram_tensor("x0T", [D, 128], bf16, kind="ExternalInput")
    nc.tensors["wq8"] = nc.dram_tensor("wq8", [D, A], fp8, kind="ExternalInput")
    nc.tensors["wk8"] = nc.dram_tensor("wk8", [D, A], fp8, kind="ExternalInput")
    nc.tensors["wv8"] = nc.dram_tensor("wv8", [D, A], fp8, kind="ExternalInput")
    nc.tensors["wq16"] = nc.dram_tensor("wq16", [D, A], bf16, kind="ExternalInput")
    nc.tensors["wk16"] = nc.dram_tensor("wk16", [D, A], bf16, kind="ExternalInput")
    nc.tensors["wv16"] = nc.dram_tensor("wv16", [D, A], bf16, kind="ExternalInput")
    nc.tensors["bq"] = nc.dram_tensor("bq", [A], f32, kind="ExternalInput")
    nc.tensors["bk"] = nc.dram_tensor("bk", [A], f32, kind="ExternalInput")
    nc.tensors["bv"] = nc.dram_tensor("bv", [A], f32, kind="ExternalInput")
    nc.tensors["mask"] = nc.dram_tensor("mask", [128, 128], f32, kind="ExternalInput")
    nc.tensors["ones2"] = nc.dram_tensor("ones2", [2], f32, kind="ExternalInput")
    nc.tensors["out"] = nc.dram_tensor("out", [S, A], f32, kind="ExternalOutput")

    with tile.TileContext(nc) as tc:
        if repeat > 1:
            with tc.For_i(0, repeat, 1):
                _emit_body(nc, tc)
        else:
            _emit_body(nc, tc)

    nc.finalize()
    return nc


def _prep_in_maps(x, Wq, bq, Wk, bk, Wv, bv):
    """Build per-core input maps (host-side shard + layout/dtype transforms)."""
    import ml_dtypes

    bf = ml_dtypes.bfloat16
    f8 = ml_dtypes.float8_e4m3  # TRN FP8_EXP4 (max +-240, has inf) == this dtype
    x = np.asarray(x, dtype=np.float32)
    wqT = np.ascontiguousarray(np.asarray(Wq, dtype=np.float32).T)
    wkT = np.ascontiguousarray(np.asarray(Wk, dtype=np.float32).T)
    wvT = np.ascontiguousarray(np.asarray(Wv, dtype=np.float32).T)
    bq = np.asarray(bq, dtype=np.float32)
    bk = np.asarray(bk, dtype=np.float32)
    bv = np.asarray(bv, dtype=np.float32)
    kq = np.arange(128)
    mask = np.where(kq[:, None] <= kq[None, :], 0.0, NEG).astype(np.float32)
    ones2 = np.ones(2, dtype=np.float32)
    shared = {
        "wq8": wqT.astype(f8), "wk8": wkT.astype(f8), "wv8": wvT.astype(f8),
        "wq16": wqT.astype(bf), "wk16": wkT.astype(bf), "wv16": wvT.astype(bf),
        "bq": bq, "bk": bk, "bv": bv, "mask": mask, "ones2": ones2,
    }
    in_maps = []
    for c in range(B):
        xTc = np.ascontiguousarray(x[:, c, :].T)  # (D, S) f32
        in_maps.append(
            {"xT8": xTc.astype(f8), "x0T": xTc[:, 0:128].astype(bf), **shared}
        )
    return in_maps


def get_nc(repeat=1):
    key = ("nc", repeat)
    if key not in _cache:
        _cache[key] = _build(repeat)
    return _cache[key]


def kernel(x, Wq, bq, Wk, bk, Wv, bv):
    from concourse.bass_utils import run_bass_kernel_spmd

    nc = get_nc()
    in_maps = _prep_in_maps(x, Wq, bq, Wk, bk, Wv, bv)
    res = run_bass_kernel_spmd(nc, in_maps, core_ids=list(range(B)))
    outs = np.stack([res.results[c]["out"] for c in range(B)], axis=0)  # (B, S, A)
    return np.ascontiguousarray(outs.transpose(1, 0, 2))  # (S, B, A)


# revision 13
# speedup vs baseline: 1.1952x; 1.1832x over previous
"""Causal single-head attention on 8 trn2 NeuronCores, data-parallel over batch.

Reference computation (per batch element b):
  Q = x_b @ Wq.T + bq ; K = x_b @ Wk.T + bk ; V = x_b @ Wv.T + bv    (S=2048, D=A=1024)
  out_b = softmax(causal(Q K^T / 32)) V

Sharding: x is (S, B=8, D); core c handles batch element c. No collectives.

Per-core kernel design (v3, fp8 DoubleRow datapath + bf16 island):
  - the whole datapath (Q/K/V projections, score matmuls, PV matmuls) runs in
    fp8(e4m3) with perf_mode=DoubleRow, which packs two 128-row contraction
    chunks per PE pass (~1.7x bf16 matmul throughput). All operands are stored
    as [128, 2, free] pair-tiles so both matmul operands present the 3D
    [Ki, 2, free] access pattern DoubleRow requires.
  - fp8 quantization noise in Q/K/V enters the output through the softmax
    average; it decays like 1/sqrt(attention window). Measured profile: rel
    err 5e-2 at q=0 falling below 7e-3 for q>=128. So query rows 0-127 (which
    attend to keys 0-127 only) are recomputed exactly in bf16 (the "island"):
    bf16 copies of W and xT[:, :128] produce exact K0/Q0/V0, a 128x128
    masked softmax and a tiny PV give rows 0-127. Cost ~12us of the ~175us
    PE budget; keeps max-norm rel err ~7e-3 (tolerance 2e-2).
  - attention runs in scores-transposed orientation ST[k, q] (stationary
    KT a-pair chunks, moving QT) so exp(ST) chunks are directly the
    stationary operand for PV and no transposes exist anywhere. The causal
    triangle is walked in 4 query-windows of 512; within a window each key
    chunk emits only its valid query columns (exact 53% triangle, no waste).
    The diagonal 128x128 sub-block gets a triangular -1e30 mask add before
    exp; exp(ST) chunks land in fp8 P pair-tiles.
  - PV contracts key pairs with DoubleRow; odd pair tails are handled by
    zero-filling the (never-written) dead 128x128 P region so every chain is
    pure DoubleRow.
  - softmax denominator: DVE accumulates the same fp8 P values into
    den[128, 512] (consistency: out = sum(P~ V)/sum(P~) cancels common-mode
    quantization), a tiny ones-matmul reduces the partition dim, and the
    reciprocal is folded into the PSUM->SBUF output evacuation scale.
  - exp has no row-max subtraction: scores are N(0, ~0.33), |s| < ~2.
  - software pipeline with a 1-window skew: ST(w+1) is emitted before PV(w)
    so the PE has independent work over the exp-latency seam.
"""
import numpy as np

S = 2048
D = 1024
A = 1024
B = 8
QW = 512            # attention query-window
NW = S // QW        # 4
KC = 128            # key-chunk
SCALE = 1.0 / 32.0  # 1/sqrt(A)
NEG = -1e30

_cache = {}


def _emit_body(nc, tc):
    import concourse.bass as bass
    import concourse.mybir as mybir

    f32 = mybir.dt.float32
    bf16 = mybir.dt.bfloat16
    fp8 = mybir.dt.float8e4
    AF = mybir.ActivationFunctionType
    DR = mybir.MatmulPerfMode.DoubleRow

    xT8, x0T = nc.tensors["xT8"], nc.tensors["x0T"]
    wq8, wk8, wv8 = nc.tensors["wq8"], nc.tensors["wk8"], nc.tensors["wv8"]
    wq16, wk16, wv16 = nc.tensors["wq16"], nc.tensors["wk16"], nc.tensors["wv16"]
    bq, bk, bv = nc.tensors["bq"], nc.tensors["bk"], nc.tensors["bv"]
    mask, ones2, out = nc.tensors["mask"], nc.tensors["ones2"], nc.tensors["out"]

    def bcast_ap(handle, n_part, n_free):
        ap = handle[:]
        return bass.AP(tensor=ap.tensor, offset=ap.offset, ap=[[0, n_part], [1, n_free]])

    with (
        tc.tile_pool(name="const", bufs=1) as cp,
        tc.tile_pool(name="kt", bufs=4) as ktp,
        tc.tile_pool(name="qt", bufs=4) as qtp,
        tc.tile_pool(name="v", bufs=8) as vp,
        tc.tile_pool(name="xt", bufs=4) as xtp,
        tc.tile_pool(name="w8", bufs=2) as wp,
        tc.tile_pool(name="w16", bufs=3) as wp16,
        tc.tile_pool(name="isl", bufs=1) as ip,
    ):
        kt = [ktp.tile([128, 2, S], fp8, tag="kt", name=f"kt{j}") for j in range(4)]
        qt = [qtp.tile([128, 2, S], fp8, tag="qt", name=f"qt{j}") for j in range(4)]
        v8 = [vp.tile([128, 2, A], fp8, tag="v", name=f"v{p}") for p in range(8)]
        xt = [xtp.tile([128, 2, S], fp8, tag="xt", name=f"xt{p}") for p in range(4)]

        # startup: narrow first wave (just the a=0 weight columns + first x
        # slice) so the first accumulation chain starts ASAP; K-weights on the
        # sync queue, x on the scalar queue (separate HWDGE pipelines)
        # tiny constants first (biases feed the first evacuations)
        bq_t = cp.tile([128, 8], f32, tag="bq")
        bk_t = cp.tile([128, 8], f32, tag="bk")
        for a in range(8):
            nc.gpsimd.dma_start(
                out=bk_t[:, a : a + 1],
                in_=bk[a * 128 : (a + 1) * 128].rearrange("(p one) -> p one", one=1),
            )
        wk8t = wp.tile([128, 8, A], fp8, tag="w", name="wk8t")
        for d in range(8):
            nc.sync.dma_start(
                out=wk8t[:, d, 0:128], in_=wk8[d * 128 : (d + 1) * 128, 0:128]
            )
            # x waves split across two queues to halve head latency
            eng = nc.scalar if d % 2 == 0 else nc.gpsimd
            eng.dma_start(
                out=xt[d // 2][:, d % 2, 0:512],
                in_=xT8[d * 128 : (d + 1) * 128, 0:512],
            )
        for d in range(8):
            nc.sync.dma_start(
                out=wk8t[:, d, 128:1024], in_=wk8[d * 128 : (d + 1) * 128, 128:1024]
            )
        for d in range(8):
            eng = nc.scalar if d % 2 == 0 else nc.gpsimd
            eng.dma_start(
                out=xt[d // 2][:, d % 2, 512:2048],
                in_=xT8[d * 128 : (d + 1) * 128, 512:2048],
            )

        for a in range(8):
            nc.gpsimd.dma_start(
                out=bq_t[:, a : a + 1],
                in_=bq[a * 128 : (a + 1) * 128].rearrange("(p one) -> p one", one=1),
            )
        bv_t = cp.tile([128, A], f32, tag="bv")
        nc.gpsimd.dma_start(out=bv_t, in_=bcast_ap(bv, 128, A))
        ones_t = cp.tile([128, 2], f32, tag="ones")
        nc.gpsimd.dma_start(out=ones_t, in_=bcast_ap(ones2, 128, 2))
        mask_t = cp.tile([128, 128], f32, tag="mask")
        nc.gpsimd.dma_start(out=mask_t, in_=mask[:, :])
        ones_b = cp.tile([128, 2], bf16, tag="onesb")
        nc.vector.tensor_copy(ones_b, ones_t)

        # island inputs (needed ~90us in; gpsimd + scalar queue tails)
        x0t = [ip.tile([128, 128], bf16, tag=f"x0_{d}", name=f"x0_{d}") for d in range(8)]
        for d in range(8):
            nc.gpsimd.dma_start(out=x0t[d], in_=x0T[d * 128 : (d + 1) * 128, :])
        wk16t = wp16.tile([128, 8, A], bf16, tag="w16", name="wk16t")
        wq16t = wp16.tile([128, 8, A], bf16, tag="w16", name="wq16t")
        wv16t = wp16.tile([128, 8, A], bf16, tag="w16", name="wv16t")
        for d in range(8):
            nc.gpsimd.dma_start(out=wk16t[:, d, :], in_=wk16[d * 128 : (d + 1) * 128, :])
        for d in range(8):
            nc.gpsimd.dma_start(out=wq16t[:, d, :], in_=wq16[d * 128 : (d + 1) * 128, :])
        for d in range(8):
            nc.scalar.dma_start(out=wv16t[:, d, :], in_=wv16[d * 128 : (d + 1) * 128, :])

        # ---- fp8 projections: 4 DoubleRow d-pair passes per PSUM chain ----
        proj_psp = tc.tile_pool(name="ps", bufs=4, space="PSUM")
        psp = proj_psp.__enter__()

        def proj_qk(w8t, bias_t, dest, skip0=0):
            # skip0: omit the first `skip0` s-columns (qt cols 0:128 are only
            # ever consumed by the bf16 island, never by the fp8 path)
            for s4 in range(4):
                slo = s4 * 512 + (skip0 if s4 == 0 else 0)
                shi = (s4 + 1) * 512
                for a in range(8):
                    ps = psp.tile([128, shi - slo], f32, tag="ps")
                    for p in range(4):
                        nc.tensor.matmul(
                            ps,
                            w8t[:, 2 * p : 2 * p + 2, a * 128 : (a + 1) * 128],
                            xt[p][:, :, slo:shi],
                            start=(p == 0),
                            stop=(p == 3),
                            perf_mode=DR,
                        )
                    nc.scalar.activation(
                        dest[a // 2][:, a % 2, slo:shi],
                        ps,
                        AF.Identity,
                        bias=bias_t[:, a : a + 1],
                    )

        proj_qk(wk8t, bk_t, kt)

        wq8t = wp.tile([128, 8, A], fp8, tag="w", name="wq8t")
        for d in range(8):
            nc.sync.dma_start(out=wq8t[:, d, :], in_=wq8[d * 128 : (d + 1) * 128, :])
        proj_qk(wq8t, bq_t, qt, skip0=128)

        # ---- island projections (bf16, exact K0/Q0/V0 for rows 0-127) ----
        # K0/Q0 are computed in the fast [s, a] orientation (x0 chunks
        # stationary, W16 moving, N=512 chains: 4x fewer PE passes than the
        # [a, s] orientation's N=128 chains), the per-a bias is added on DVE
        # from broadcast tiles, and DMA XBAR transposes (off-PE) flip each
        # 128x128 block into the [a, s] layout the score matmuls need.
        bkb_t = cp.tile([128, A], f32, tag="bkb")
        nc.gpsimd.dma_start(out=bkb_t, in_=bcast_ap(bk, 128, A))
        bqb_t = cp.tile([128, A], f32, tag="bqb")
        nc.gpsimd.dma_start(out=bqb_t, in_=bcast_ap(bq, 128, A))
        kt0 = [ip.tile([128, 128], bf16, tag=f"kt0_{a}", name=f"kt0_{a}") for a in range(8)]
        qt0 = [ip.tile([128, 128], bf16, tag=f"qt0_{a}", name=f"qt0_{a}") for a in range(8)]
        k0sa = ip.tile([128, A], bf16, tag="k0sa")
        q0sa = ip.tile([128, A], bf16, tag="q0sa")
        v0 = ip.tile([128, A], bf16, tag="v0")
        for wt16, bb, dst in ((wk16t, bkb_t, k0sa), (wq16t, bqb_t, q0sa)):
            for ah in range(2):
                ps = psp.tile([128, 512], f32, tag="ps", name=f"pskq0_{ah}")
                for d in range(8):
                    nc.tensor.matmul(
                        ps, x0t[d], wt16[:, d, ah * 512 : (ah + 1) * 512],
                        start=(d == 0), stop=(d == 7),
                    )
                nc.vector.tensor_add(
                    dst[:, ah * 512 : (ah + 1) * 512], ps, bb[:, ah * 512 : (ah + 1) * 512]
                )
        for a in range(8):
            nc.scalar.dma_start_transpose(kt0[a], k0sa[:, a * 128 : (a + 1) * 128])
            nc.scalar.dma_start_transpose(qt0[a], q0sa[:, a * 128 : (a + 1) * 128])

        wv8t = wp.tile([128, 8, A], fp8, tag="w", name="wv8t")
        for d in range(8):
            nc.sync.dma_start(out=wv8t[:, d, :], in_=wv8[d * 128 : (d + 1) * 128, :])
        for sc in range(16):
            for ah in range(2):
                ps = psp.tile([128, 512], f32, tag="ps")
                for p in range(4):
                    nc.tensor.matmul(
                        ps,
                        xt[p][:, :, sc * 128 : (sc + 1) * 128],
                        wv8t[:, 2 * p : 2 * p + 2, ah * 512 : (ah + 1) * 512],
                        start=(p == 0),
                        stop=(p == 3),
                        perf_mode=DR,
                    )
                nc.vector.tensor_add(
                    v8[sc // 2][:, sc % 2, ah * 512 : (ah + 1) * 512],
                    ps,
                    bv_t[:, ah * 512 : (ah + 1) * 512],
                )

        for ah in range(2):
            ps = psp.tile([128, 512], f32, tag="ps", name=f"psv0_{ah}")
            for d in range(8):
                nc.tensor.matmul(
                    ps, x0t[d], wv16t[:, d, ah * 512 : (ah + 1) * 512],
                    start=(d == 0), stop=(d == 7),
                )
            nc.vector.tensor_add(
                v0[:, ah * 512 : (ah + 1) * 512], ps, bv_t[:, ah * 512 : (ah + 1) * 512]
            )

        proj_psp.__exit__(None, None, None)

        # ---- attention ----
        with (
            tc.tile_pool(name="p8", bufs=16) as pp,
            tc.tile_pool(name="stt", bufs=3) as sttp,
            tc.tile_pool(name="den", bufs=2) as dnp,
            tc.tile_pool(name="ob", bufs=3) as obp,
            tc.tile_pool(name="rin", bufs=6) as rp,
            tc.tile_pool(name="pst", bufs=4, space="PSUM") as pstp,
            tc.tile_pool(name="po", bufs=3, space="PSUM") as pop,
            tc.tile_pool(name="pd", bufs=1, space="PSUM") as pdp,
        ):
            def island_scores():
                ps = pstp.tile([128, 128], f32, tag="st", name="ps0")
                for a in range(8):
                    nc.tensor.matmul(ps, kt0[a], qt0[a], start=(a == 0), stop=(a == 7))
                stt = sttp.tile([128, 128], f32, tag="stt", name="stt0")
                nc.vector.tensor_add(stt, ps, mask_t)
                p0 = ip.tile([128, 128], bf16, tag="p0")
                nc.scalar.activation(p0, stt, AF.Exp, scale=SCALE)
                return p0

            def island_pv(p0):
                pd = pdp.tile([128, 2], f32, tag="pd", name="pd0")
                nc.tensor.matmul(pd, p0, ones_b, start=True, stop=True)
                rinv = rp.tile([128, 1], f32, tag="rinv", name="rinv0")
                nc.vector.reciprocal(rinv, pd[:, 0:1])
                osb = obp.tile([128, A], f32, tag="ob", name="osb0")
                for ah in range(2):
                    po = pop.tile([128, 512], f32, tag="po", name=f"po0_{ah}")
                    nc.tensor.matmul(
                        po, p0, v0[:, ah * 512 : (ah + 1) * 512], start=True, stop=True
                    )
                    nc.scalar.activation(
                        osb[:, ah * 512 : (ah + 1) * 512], po, AF.Copy, scale=rinv
                    )
                nc.sync.dma_start(out=out[0:128, :], in_=osb)

            def emit_st(w):
                """Score chunks + exp + denominator partials for one q-window."""
                kcmax = 4 * (w + 1)
                den = dnp.tile([128, QW], f32, tag="den", name=f"den{w}")
                nc.gpsimd.memset(den, 0.0)
                pw = [
                    pp.tile([128, 2, QW], fp8, tag="p8", name=f"p{w}_{j}")
                    for j in range(kcmax // 2)
                ]
                for kc in range(kcmax):
                    qlo_g = max(w * QW, kc * KC)
                    if w == 0 and kc == 0:
                        qlo_g = 128  # rows 0-127 belong to the bf16 island
                    ql = qlo_g - w * QW
                    nq = QW - ql
                    ps = pstp.tile([128, nq], f32, tag="st", name=f"st{w}_{kc}")
                    if nq <= 256:
                        # narrow block: DoubleRow is LDWEIGHTS-bound here, plain
                        # fp8 128-contraction MMs with FWL weight loads win
                        for j in range(4):
                            for i in range(2):
                                nc.tensor.matmul(
                                    ps,
                                    kt[j][:, i, kc * KC : (kc + 1) * KC],
                                    qt[j][:, i, qlo_g : (w + 1) * QW],
                                    start=(j == 0 and i == 0),
                                    stop=(j == 3 and i == 1),
                                )
                    else:
                        for j in range(4):
                            nc.tensor.matmul(
                                ps,
                                kt[j][:, :, kc * KC : (kc + 1) * KC],
                                qt[j][:, :, qlo_g : (w + 1) * QW],
                                start=(j == 0),
                                stop=(j == 3),
                                perf_mode=DR,
                            )
                    sub = kc % 2
                    pwt = pw[kc // 2]
                    if kc // 4 == w and kc != 0:
                        # diagonal 128x128 sub-block: triangular causal mask
                        stt = sttp.tile([128, 128], f32, tag="stt", name=f"stt{w}_{kc}")
                        nc.vector.tensor_add(stt, ps[:, 0:128], mask_t)
                        nc.scalar.activation(
                            pwt[:, sub, ql : ql + 128], stt, AF.Exp, scale=SCALE
                        )
                        if nq > 128:
                            nc.scalar.activation(
                                pwt[:, sub, ql + 128 : QW], ps[:, 128:nq],
                                AF.Exp, scale=SCALE,
                            )
                    else:
                        nc.scalar.activation(pwt[:, sub, ql:QW], ps, AF.Exp, scale=SCALE)
                    nc.vector.tensor_add(
                        den[:, ql:QW], den[:, ql:QW], pwt[:, sub, ql:QW]
                    )
                return pw, den

            def emit_pv(w, pw, den):
                """Denominator reduction + PV chains + evacuation for one q-window."""
                for qc in range(4 * w, 4 * w + 4):
                    if qc == 0:
                        continue
                    ql = (qc % 4) * 128
                    nfull = (qc + 1) // 2  # full k-pairs
                    odd = (qc + 1) % 2     # trailing single k-chunk
                    pd = pdp.tile([128, 2], f32, tag="pd", name=f"pd{qc}")
                    nc.tensor.matmul(
                        pd, den[:, ql : ql + 128], ones_t, start=True, stop=True
                    )
                    rinv = rp.tile([128, 1], f32, tag="rinv", name=f"rinv{qc}")
                    nc.vector.reciprocal(rinv, pd[:, 0:1])
                    osb = obp.tile([128, A], f32, tag="ob", name=f"ob{qc}")
                    for ah in range(2):
                        po = pop.tile([128, 512], f32, tag="po", name=f"po{qc}_{ah}")
                        for p in range(nfull):
                            nc.tensor.matmul(
                                po,
                                pw[p][:, :, ql : ql + 128],
                                v8[p][:, :, ah * 512 : (ah + 1) * 512],
                                start=(p == 0),
                                stop=(p == nfull - 1 and not odd),
                                perf_mode=DR,
                            )
                        if odd:
                            nc.tensor.matmul(
                                po,
                                pw[nfull][:, 0, ql : ql + 128],
                                v8[nfull][:, 0, ah * 512 : (ah + 1) * 512],
                                start=(nfull == 0),
                                stop=True,
                            )
                        nc.scalar.activation(
                            osb[:, ah * 512 : (ah + 1) * 512], po, AF.Copy, scale=rinv
                        )
                        nc.sync.dma_start(
                            out=out[qc * 128 : (qc + 1) * 128, ah * 512 : (ah + 1) * 512],
                            in_=osb[:, ah * 512 : (ah + 1) * 512],
                        )

            # software pipeline with a 1-window skew: ST(w+1) is emitted before
            # PV(w) so the PE has independent work over the exp-latency seam
            p0 = island_scores()
            prev = emit_st(0)
            island_pv(p0)
            for w in range(1, NW):
                st_state = emit_st(w)
                emit_pv(w - 1, *prev)
                prev = st_state
            emit_pv(NW - 1, *prev)


def _build(repeat=1):
    from concourse import bacc
    import concourse.mybir as mybir
    import concourse.tile as tile

    f32 = mybir.dt.float32
    bf16 = mybir.dt.bfloat16
    fp8 = mybir.dt.float8e4

    nc = bacc.Bacc("TRN2", target_bir_lowering=False)
    nc.tensors = {}
    nc.tensors["xT8"] = nc.dram_tensor("xT8", [D, S], fp8, kind="ExternalInput")
    nc.tensors["x0T"] = nc.dram_tensor("x0T", [D, 128], bf16, kind="ExternalInput")
    nc.tensors["wq8"] = nc.dram_tensor("wq8", [D, A], fp8, kind="ExternalInput")
    nc.tensors["wk8"] = nc.dram_tensor("wk8", [D, A], fp8, kind="ExternalInput")
    nc.tensors["wv8"] = nc.dram_tensor("wv8", [D, A], fp8, kind="ExternalInput")
    nc.tensors["wq16"] = nc.dram_tensor("wq16", [D, A], bf16, kind="ExternalInput")
    nc.tensors["wk16"] = nc.dram_tensor("wk16", [D, A], bf16, kind="ExternalInput")
    nc.tensors["wv16"] = nc.dram_tensor("wv16", [D, A], bf16, kind="ExternalInput")
    nc.tensors["bq"] = nc.dram_tensor("bq", [A], f32, kind="ExternalInput")
    nc.tensors["bk"] = nc.dram_tensor("bk", [A], f32, kind="ExternalInput")
    nc.tensors["bv"] = nc.dram_tensor("bv", [A], f32, kind="ExternalInput")
    nc.tensors["mask"] = nc.dram_tensor("mask", [128, 128], f32, kind="ExternalInput")
    nc.tensors["ones2"] = nc.dram_tensor("ones2", [2], f32, kind="ExternalInput")
    nc.tensors["out"] = nc.dram_tensor("out", [S, A], f32, kind="ExternalOutput")

    with tile.TileContext(nc) as tc:
        if repeat > 1:
            with tc.For_i(0, repeat, 1):
                _emit_body(nc, tc)
        else:
            _emit_body(nc, tc)

    nc.finalize()
    return nc


def _prep_in_maps(x, Wq, bq, Wk, bk, Wv, bv):
    """Build per-core input maps (host-side shard + layout/dtype transforms)."""
    import ml_dtypes

    bf = ml_dtypes.bfloat16
    f8 = ml_dtypes.float8_e4m3  # TRN FP8_EXP4 (max +-240, has inf) == this dtype
    x = np.asarray(x, dtype=np.float32)
    wqT = np.ascontiguousarray(np.asarray(Wq, dtype=np.float32).T)
    wkT = np.ascontiguousarray(np.asarray(Wk, dtype=np.float32).T)
    wvT = np.ascontiguousarray(np.asarray(Wv, dtype=np.float32).T)
    bq = np.asarray(bq, dtype=np.float32)
    bk = np.asarray(bk, dtype=np.float32)
    bv = np.asarray(bv, dtype=np.float32)
    kq = np.arange(128)
    mask = np.where(kq[:, None] <= kq[None, :], 0.0, NEG).astype(np.float32)
    ones2 = np.ones(2, dtype=np.float32)
    shared = {
        "wq8": wqT.astype(f8), "wk8": wkT.astype(f8), "wv8": wvT.astype(f8),
        "wq16": wqT.astype(bf), "wk16": wkT.astype(bf), "wv16": wvT.astype(bf),
        "bq": bq, "bk": bk, "bv": bv, "mask": mask, "ones2": ones2,
    }
    in_maps = []
    for c in range(B):
        xTc = np.ascontiguousarray(x[:, c, :].T)  # (D, S) f32
        in_maps.append(
            {"xT8": xTc.astype(f8), "x0T": xTc[:, 0:128].astype(bf), **shared}
        )
    return in_maps


def get_nc(repeat=1):
    key = ("nc", repeat)
    if key not in _cache:
        _cache[key] = _build(repeat)
    return _cache[key]


def kernel(x, Wq, bq, Wk, bk, Wv, bv):
    from concourse.bass_utils import run_bass_kernel_spmd

    nc = get_nc()
    in_maps = _prep_in_maps(x, Wq, bq, Wk, bk, Wv, bv)
    res = run_bass_kernel_spmd(nc, in_maps, core_ids=list(range(B)))
    outs = np.stack([res.results[c]["out"] for c in range(B)], axis=0)  # (B, S, A)
    return np.ascontiguousarray(outs.transpose(1, 0, 2))  # (S, B, A)
